# revision 33
# baseline (speedup 1.0000x reference)
"""Trainium2 Bass kernel for nn_AdvancedCRSN (vq_codebook).

Contract: kernel(**inputs) -> (logits [B,32] f32, ponder scalar, vq_total scalar)
matching reference.py's forward pass (up to fp tolerance).

Strategy:
  - Pure data parallel across 8 NeuronCores (B=131072 -> 16384 tokens/core).
  - Host precomputes the complex embedding z0 = (r*cos t, r*sin t) and fused
    parameter matrices; device runs the 8 recurrent steps.
  - Layout: features on partitions ([128, 512] tiles = 512 tokens on the free
    dim).  Cross-feature reductions (LN stats, VQ scores, zq gather, decode)
    are PE matmuls with stationary weights; the VQ argmax uses DVE 32x32
    block transposes + a free-dim max reduce.
  - Loop order: tile-outer / step-inner, so the decode matmul accumulates
    per-tile in one PSUM bank across all 8 steps.
  - Forward-pass identities used:
      * straight-through zq_st == zq; vq_loss == 1.25*mean((zq-feats)^2)
      * argmin_s d == argmax_s (cb_s.f + 0.5*GB*sig(adj[sym])_s - 0.5|cb_s|^2)
      * the graph-bias matmul over onehot_prev also folds -0.5|cb|^2
        (onehot sums to 1); step 0 uses a ones-row rhs instead.
      * modrelu is exact identity when mr_bias == 0; the LN affine is
        identity when ln_scale == 1, ln_shift == 0 (checked at runtime,
        generic ops emitted otherwise).
  - This walrus build requires all compute-op INPUT operands to share a
    start partition (outputs may shift by multiples of 32), allows at most
    one sync-wait on ctrl-class instructions (handled by a BIR post-pass),
    and cannot compile GPSIMD compute ops.
"""

import os
import sys

import numpy as np

for p in ("/opt/trn_rl_repo",):
    if p not in sys.path and os.path.isdir(p):
        sys.path.insert(0, p)

import concourse.bass as bass
import concourse.tile as tile
from concourse import mybir
from concourse.bass_types import AP
from concourse.bass_utils import run_bass_kernel_spmd

F32 = mybir.dt.float32
F32R = mybir.dt.float32r
AF = mybir.ActivationFunctionType
OP = mybir.AluOpType
AX = mybir.AxisListType

EPS = 1e-6
D = 64
NS = 32
VOCAB = 32
DEPTH = 8
ACT_TH = 0.99
GB_SCALE = 0.1
CC = 0.25
B = 131072
NCORES = 8
BC = B // NCORES          # tokens per core
NT = 512                  # tokens per tile (free dim)
NTILES = BC // NT
LAST_EXEC_NS = None


def _r(ap):
    """matmul operand dtype: plain fp32 (this walrus requires fp32r inputs
    to be explicitly pre-rounded, and PE is not the bottleneck here)"""
    return ap


def _bcast_last(src, n):
    """AP reading src [P, F] as [P, F, n] with stride-0 innermost."""
    return AP(tensor=src.tensor, offset=src.offset,
              ap=list(src.ap) + [[0, n]])


def _split_ctrl_waits(bir_bytes, maxw=1, ctrl_ops=("Drain", "EventSemaphore")):
    """This walrus accepts only 1 sync-wait on ctrl-class (no-struct)
    instructions; distribute extras onto inserted same-engine drains."""
    import json as _json
    d = _json.loads(bir_bytes)
    uid = [0]
    for fn in d["functions"]:
        for bb in fn["blocks"]:
            out = []
            for ins in bb["instructions"]:
                si = ins.get("sync_info") or {}
                w = si.get("on_wait") or []
                lim = maxw
                if len(w) > lim:
                    chunks = [w[i:i + lim] for i in range(0, len(w), lim)]
                    for ch in chunks[:-1]:
                        uid[0] += 1
                        carrier = {
                            "engine": ins.get("engine"),
                            "ins": [], "outs": [], "instr": [],
                            "name": f"I-ws{uid[0]}",
                            "opcode": "Drain",
                            "sync_info": {"on_update": [], "on_wait": ch},
                        }
                        if "debug" in ins:
                            carrier["debug"] = ins["debug"]
                        out.append(carrier)
                    si = dict(si)
                    si["on_wait"] = chunks[-1]
                    ins = dict(ins)
                    ins["sync_info"] = si
                out.append(ins)
            bb["instructions"] = out
    return _json.dumps(d).encode()


def _install_bir_fixup(nc):
    orig = nc.to_json_bytes
    nc.to_json_bytes = lambda: _split_ctrl_waits(orig())
    return nc


def build_program(ln_trivial, mr_trivial):
    nc = bass.Bass(target_bir_lowering=False, debug=False)

    # ---- DRAM I/O ----
    z0_d = nc.declare_dram_parameter("z0", [128, BC], F32, isOutput=False)
    w2_d = nc.declare_dram_parameter("w2", [128, 128], F32, isOutput=False)
    cbh_d = nc.declare_dram_parameter("cbh", [128, NS + 1], F32, isOutput=False)
    adj2_d = nc.declare_dram_parameter("adj2", [NS, NS + 1], F32, isOutput=False)
    cb0_d = nc.declare_dram_parameter("cb0", [1, NS + 1], F32, isOutput=False)
    cb03_d = nc.declare_dram_parameter("cb03", [NS, 128], F32, isOutput=False)
    dec_d = nc.declare_dram_parameter("dect", [128, VOCAB], F32, isOutput=False)
    aux_d = nc.declare_dram_parameter("aux", [128, 8], F32, isOutput=False)
    ocst_d = nc.declare_dram_parameter("ocst", [128, 832], F32, isOutput=False)
    idm_d = nc.declare_dram_parameter("idm", [128, 128], F32, isOutput=False)
    # aux cols: 0 ln_scale, 1 ln_shift, 2 mr_bias+EPS, 3 unused, 4 EPS, 5 halt_b
    logits_d = nc.declare_dram_parameter("logits", [VOCAB, BC], F32,
                                         isOutput=True)
    scal_d = nc.declare_dram_parameter("scalars", [1, 2], F32, isOutput=True)

    NSLOT = DEPTH * NTILES

    from contextlib import ExitStack

    with tile.TileContext(nc) as tc, ExitStack() as ctx:
        ctx.enter_context(nc.allow_low_precision(
            reason="float32r outs feed fp32r matmuls; same 4-byte precision"))
        const = ctx.enter_context(tc.tile_pool(name="const", bufs=1))
        state = ctx.enter_context(tc.tile_pool(name="state", bufs=1))
        work = ctx.enter_context(tc.tile_pool(name="work", bufs=2))
        pers = ctx.enter_context(tc.tile_pool(name="pers", bufs=2))
        ps1 = ctx.enter_context(tc.tile_pool(name="ps1", bufs=1, space="PSUM"))

        # ---- constants to SBUF ----
        w2 = const.tile([128, 128], F32)
        nc.sync.dma_start(out=w2, in_=w2_d[:, :])
        cbh = const.tile([128, NS + 1], F32)
        nc.sync.dma_start(out=cbh, in_=cbh_d[:, :])
        adj2 = const.tile([NS, NS + 1], F32)
        nc.sync.dma_start(out=adj2, in_=adj2_d[:, :])
        cb0 = const.tile([1, NS + 1], F32)
        nc.sync.dma_start(out=cb0, in_=cb0_d[:, :])
        cb03 = const.tile([NS, 128], F32)
        nc.sync.dma_start(out=cb03, in_=cb03_d[:, :])
        dect = const.tile([128, VOCAB], F32)
        nc.sync.dma_start(out=dect, in_=dec_d[:, :])
        aux = const.tile([128, 8], F32)
        nc.sync.dma_start(out=aux, in_=aux_d[:, :])

        ones64_64 = const.tile([D, D], F32)      # 1/64 -> meanB
        nc.vector.memset(ones64_64, 1.0 / D)
        ones64_1 = const.tile([D, 1], F32)
        nc.vector.memset(ones64_1, 1.0)
        ones1_64 = const.tile([1, D], F32)
        nc.vector.memset(ones1_64, 1.0)
        ones1_128 = const.tile([1, 128], F32)
        nc.vector.memset(ones1_128, 1.0)
        ones128_1 = const.tile([128, 1], F32)
        nc.vector.memset(ones128_1, 1.0)
        const1 = const.tile([1, NT], F32)
        nc.vector.memset(const1, 1.0)
        lowc = const.tile([128, NT], F32)    # -inf-ish for max-based realigns
        nc.vector.memset(lowc, -1.0e30)
        ones128f = const.tile([128, 1], F32)
        nc.vector.memset(ones128f, 1.0)

        pond = state.tile([1, NSLOT], F32)
        nc.vector.memset(pond, 0.0)
        vqp = state.tile([128, NSLOT], F32)
        scal = state.tile([1, 2], F32)

        for t in range(NTILES):
            # f stash: slice s holds the INPUT feats of step s (slice 0 = z0)
            fs = pers.tile([128, (DEPTH + 1) * NT], F32, tag="fs")
            nc.sync.dma_start(out=fs[:, 0:NT], in_=z0_d[:, t * NT:(t + 1) * NT])
            hs = pers.tile([33, DEPTH * NT], F32, tag="hs")
            oh = None

            for s in range(DEPTH):
                idx = s * NTILES + t
                f = fs[:, s * NT:(s + 1) * NT]

                # complex linear  z = W2^T f -> [nr; ni] in PSUM
                psz = ps2.tile([128, NT], F32, tag="z")
                nc.tensor.matmul(psz, lhsT=_r(w2), rhs=_r(f), start=True,
                                 stop=True)

                # realign ni to partitions 0:64, then magnitudes
                ni0 = work.tile([D, NT], F32, tag="ni0")
                nc.vector.tensor_tensor(ni0, psz[D:128], lowc[D:128], op=OP.max)
                sqr = work.tile([D, NT], F32, tag="sqr")
                nc.scalar.activation(sqr, psz[0:D], AF.Square)
                sqi = work.tile([D, NT], F32, tag="sqi")
                nc.scalar.activation(sqi, ni0, AF.Square)
                m2 = work.tile([D, NT], F32, tag="m2")
                nc.vector.tensor_tensor(m2, sqr, sqi, op=OP.add)
                raw = work.tile([D, NT], F32, tag="raw")
                nc.scalar.activation(raw, m2, AF.Sqrt)

                # LN stats: meanB broadcast via all-ones matmul; s2 = sum(m2)
                psmr = ps1.tile([D, NT], F32, tag="mr")
                nc.tensor.matmul(psmr, lhsT=_r(ones64_64), rhs=_r(raw),
                                 start=True, stop=True)
                psrs = ps1.tile([D, NT], F32, tag="rs")
                nc.tensor.matmul(psrs[0:1], lhsT=_r(ones64_1), rhs=_r(m2),
                                 start=True, stop=True)
                msq = work.tile([1, NT], F32, tag="msq")
                nc.scalar.activation(msq, psmr[0:1], AF.Square,
                                     scale=float(8.0 / np.sqrt(63.0)))
                nc.vector.scalar_tensor_tensor(msq, psrs[0:1], 1.0 / 63.0, msq,
                                               op0=OP.mult, op1=OP.subtract)
                srt = work.tile([1, NT], F32, tag="srt")
                nc.scalar.activation(srt, msq, AF.Sqrt, bias=aux[0:1, 4:5])
                nc.tensor.matmul(psrs, lhsT=_r(ones1_64), rhs=_r(srt),
                                 start=True, stop=True, skip_group_check=True)

                # z3 = (raw - mean)/(srt * raw) * z   (one reciprocal)
                cent = work.tile([D, NT], F32, tag="cent")
                nc.vector.tensor_tensor(cent, raw, psmr, op=OP.subtract)
                den = work.tile([D, NT], F32, tag="den")
                nc.vector.tensor_tensor(den, raw, psrs, op=OP.mult)
                invd = work.tile([D, NT], F32, tag="invd")
                nc.vector.reciprocal(invd, den)
                if not ln_trivial:
                    # fold ln_scale into cent; ln_shift handled below
                    nc.vector.tensor_scalar(cent, cent, aux[0:D, 0:1],
                                            None, op0=OP.mult)
                w = work.tile([D, NT], F32, tag="w")
                nc.vector.tensor_tensor(w, cent, invd, op=OP.mult)
                z3 = work.tile([128, NT], F32, tag="z3")
                nc.vector.tensor_tensor(z3[0:D], w, psz[0:D], op=OP.mult)
                nc.vector.tensor_tensor(z3[D:128], w, ni0, op=OP.mult)
                if not ln_trivial:
                    # z3 += ln_shift[f] * (z/raw):  cos/sin phase terms
                    cp = work.tile([D, NT], F32, tag="den")
                    nc.vector.tensor_tensor(cp, invd, psrs, op=OP.mult)
                    pr = work.tile([D, NT], F32, tag="ni0")
                    nc.vector.tensor_tensor(pr, cp, psz[0:D], op=OP.mult)
                    nc.vector.scalar_tensor_tensor(z3[0:D], pr, aux[0:D, 1:2],
                                                   z3[0:D], op0=OP.mult,
                                                   op1=OP.add)
                    nc.vector.tensor_tensor(pr, cp, ni0, op=OP.mult)
                    nc.vector.scalar_tensor_tensor(z3[D:128], pr,
                                                   aux[0:D, 1:2], z3[D:128],
                                                   op0=OP.mult, op1=OP.add)

                if not mr_trivial:
                    # generic modrelu: z3 *= relu(|z3|+eps+mrb)/(|z3|+eps)
                    ni2 = work.tile([D, NT], F32, tag="w")
                    nc.vector.tensor_tensor(ni2, z3[D:128], z3[D:128],
                                            op=OP.max)
                    g2 = work.tile([D, NT], F32, tag="den")
                    nc.scalar.activation(g2, z3[0:D], AF.Square)
                    g3 = work.tile([D, NT], F32, tag="raw")
                    nc.scalar.activation(g3, ni2, AF.Square)
                    nc.vector.tensor_tensor(g2, g2, g3, op=OP.add)
                    g4 = work.tile([D, NT], F32, tag="invd")
                    nc.scalar.activation(g4, g2, AF.Ln)
                    mt = work.tile([D, NT], F32, tag="msq")
                    nc.scalar.activation(mt, g4, AF.Exp, scale=0.5)
                    rl = work.tile([D, NT], F32, tag="srt")
                    nc.scalar.activation(rl, mt, AF.Relu, bias=aux[0:D, 2:3])
                    me = work.tile([D, NT], F32, tag="qm")
                    nc.vector.tensor_scalar(me, mt, float(EPS), None,
                                            op0=OP.add)
                    g5 = work.tile([D, NT], F32, tag="cent")
                    nc.scalar.activation(g5, me, AF.Ln)
                    nc.scalar.activation(me, g5, AF.Exp, scale=-1.0)
                    nc.vector.tensor_tensor(rl, rl, me, op=OP.mult)
                    nc.vector.tensor_tensor(z3[0:D], z3[0:D], rl, op=OP.mult)
                    nc.vector.tensor_tensor(z3[D:128], z3[D:128], rl,
                                            op=OP.mult)

                # VQ scores: q = cb.z3 (+halt col) [+ graph bias - .5|cb|^2]
                psqs = ps1.tile([NS + 1, NT], F32, tag="qs")
                nc.tensor.matmul(psqs, lhsT=_r(cbh), rhs=_r(z3),
                                 start=True, stop=False)
                if s == 0:
                    nc.tensor.matmul(psqs, lhsT=_r(cb0),
                                     rhs=_r(const1), start=False, stop=True)
                else:
                    nc.tensor.matmul(psqs, lhsT=_r(adj2),
                                     rhs=_r(oh), start=False, stop=True)

                # stash raw halt score (sigmoid batched in the post-phase)
                nc.scalar.activation(hs[32:33, s * NT:(s + 1) * NT],
                                     psqs[NS:NS + 1], AF.Copy)

                # argmax over the 32 symbols -> onehot [NS, NT]
                qsb = work.tile([NS, NT], F32, tag="qsb")
                nc.scalar.activation(qsb, psqs[0:NS], AF.Copy)
                oht = work.tile([128, 4, NS], F32, tag="oht")
                for c in range(4):
                    psqt = ps1.tile([128, NS], F32, tag="qs")
                    nc.tensor.transpose(psqt, qsb[:, c * 128:(c + 1) * 128],
                                        idm[0:NS, 0:NS])
                    qm = work.tile([128, 4], F32, tag="qm")
                    nc.vector.tensor_reduce(qm[:, c:c + 1], psqt, axis=AX.X,
                                            op=OP.max)
                    nc.vector.tensor_scalar(oht[:, c, :], psqt, qm[:, c:c + 1],
                                            None, op0=OP.is_ge)
                psoh = ps1.tile([NS, NT], F32, tag="qs")
                for c in range(4):
                    nc.tensor.transpose(psoh[:, c * 128:(c + 1) * 128],
                                        oht[:, c, :], idm)
                oh = work.tile([NS, NT], F32, tag="oh")
                nc.scalar.activation(oh, psoh, AF.Copy)

                # zq (x0.3) via codebook matmul
                pszq = ps1.tile([128, NT], F32, tag="zq")
                nc.tensor.matmul(pszq, lhsT=_r(cb03), rhs=_r(oh), start=True,
                                 stop=True)

                # vq loss: err = zq - z3 = pszq/0.3 - z3; accumulate err^2
                errt = work.tile([128, NT], F32, tag="errt")
                nc.vector.scalar_tensor_tensor(errt, pszq, 1.0 / 0.3, z3,
                                               op0=OP.mult, op1=OP.subtract)
                nc.scalar.activation(errt, errt, AF.Square,
                                     accum_out=vqp[:, idx:idx + 1])

                # blend: f_{s+1} = 0.7*z3 + 0.3*zq  (into the stash)
                nc.vector.scalar_tensor_tensor(
                    fs[:, (s + 1) * NT:(s + 2) * NT], z3, 0.7, pszq,
                    op0=OP.mult, op1=OP.add)

            # ---- per-tile post-phase: sigmoid (one table visit), halting,
            # peff-weighted decode accumulation ----
            nc.scalar.activation(hs[32:33, :], hs[32:33, :], AF.Sigmoid,
                                 bias=aux[32:33, 5:6])
            pall = hs
            psd = ps1.tile([VOCAB, NT], F32, tag="dec")
            h = pall[32:33, 0:NT]
            peffs = [pall[32:33, 0:NT]]
            for s in range(1, DEPTH):
                idx = s * NTILES + t
                still = work.tile([33, NT], F32, tag="still")
                nc.vector.tensor_scalar(still[32:33], h, float(ACT_TH), 0.0,
                                        op0=OP.is_lt, op1=OP.add,
                                        accum_out=pond[32:33, idx:idx + 1])
                peff = work.tile([33, NT], F32, tag=f"pf{s % 2}")
                if s < DEPTH - 1:
                    nc.vector.tensor_tensor(
                        peff[32:33], pall[32:33, s * NT:(s + 1) * NT],
                        still[32:33], op=OP.mult)
                    hn = pers.tile([33, NT], F32, tag="h")
                    nc.vector.tensor_tensor(hn[32:33], h, peff[32:33],
                                            op=OP.add)
                    h = hn[32:33]
                else:
                    nc.vector.tensor_scalar(peff[32:33], h, -1.0, 1.0,
                                            op0=OP.mult, op1=OP.add)
                peffs.append(peff[32:33])
            for s in range(DEPTH):
                pspe = ps1.tile([128, NT], F32, tag="pe")
                nc.tensor.matmul(pspe, lhsT=_r(ones1_128[32:33]),
                                 rhs=_r(peffs[s]), start=True, stop=True)
                zs = work.tile([128, NT], F32, tag="zs")
                nc.vector.tensor_tensor(zs, fs[:, (s + 1) * NT:(s + 2) * NT],
                                        pspe, op=OP.mult)
                nc.tensor.matmul(psd, lhsT=_r(dect), rhs=_r(zs),
                                 start=(s == 0), stop=(s == DEPTH - 1))
            lt = work.tile([VOCAB, NT], F32, tag="lt")
            nc.scalar.activation(lt, psd, AF.Copy)
            nc.sync.dma_start(out=logits_d[:, t * NT:(t + 1) * NT], in_=lt)

        # ---- scalar reductions ----
        vq1 = work.tile([128, 1], F32, tag="vq1")
        nc.vector.tensor_reduce(vq1, vqp, axis=AX.X, op=OP.add)
        pssc = ps1.tile([1, 1], F32, tag="pe")
        nc.tensor.matmul(pssc, lhsT=ones128f, rhs=vq1, start=True,
                         stop=True)
        nc.scalar.activation(scal[:, 0:1], pssc, AF.Copy)
        nc.vector.tensor_reduce(scal[:, 1:2], pond, axis=AX.X, op=OP.add)
        nc.sync.dma_start(out=scal_d[:, :], in_=scal)

    return _install_bir_fixup(nc)


def kernel(x, emb_mag, emb_phase, Wr, Wi, ln_scale, ln_shift, mr_bias,
           halt_w, halt_b, codebook, adj, dec_w, dec_b):
    f32 = np.float32
    x = np.asarray(x)
    xi = x[:, 0].astype(np.int64)
    r = np.asarray(emb_mag, f32)[xi]
    t = np.asarray(emb_phase, f32)[xi]
    z0r = (r * np.cos(t)).astype(f32)
    z0i = (r * np.sin(t)).astype(f32)

    Wr = np.asarray(Wr, f32)
    Wi = np.asarray(Wi, f32)
    cb = np.asarray(codebook, f32)
    adj = np.asarray(adj, f32)
    dec_w = np.asarray(dec_w, f32)
    dec_b = np.asarray(dec_b, f32)
    ln_scale = np.asarray(ln_scale, f32)
    ln_shift = np.asarray(ln_shift, f32)
    mr_bias = np.asarray(mr_bias, f32)
    halt_w = np.asarray(halt_w, f32)
    hb = float(np.asarray(halt_b, f32).reshape(-1)[0])

    # combined complex-linear weight: [nr;ni] = W2^T @ [zr;zi]
    w2 = np.zeros((128, 128), f32)
    w2[0:D, 0:D] = Wr.T
    w2[D:128, 0:D] = -Wi.T
    w2[0:D, D:128] = Wi.T
    w2[D:128, D:128] = Wr.T

    cb_sq = (cb ** 2).sum(-1).astype(f32)
    cbh = np.zeros((128, NS + 1), f32)
    cbh[:, 0:NS] = cb.T
    cbh[:, NS] = halt_w[0]
    sig_adj = (1.0 / (1.0 + np.exp(-adj.astype(np.float64)))).astype(f32)
    adj2 = np.zeros((NS, NS + 1), f32)
    adj2[:, 0:NS] = 0.5 * GB_SCALE * sig_adj - 0.5 * cb_sq[None, :]
    cb0 = np.zeros((1, NS + 1), f32)
    cb0[0, 0:NS] = -0.5 * cb_sq
    cb03 = (0.3 * cb).astype(f32)
    dect = dec_w.T.copy()
    ocst = np.zeros((128, 832), f32)
    ocst[0:D, 0:D] = 1.0 / D
    ocst[0:D, 64] = 1.0
    ocst[0, 65:129] = 1.0
    ocst[0:33, 129:257] = 1.0
    ocst[:, 257] = 1.0
    ocst[0, 320:320 + NT] = 1.0
    idm = np.eye(128, dtype=f32)
    aux = np.zeros((128, 8), f32)
    aux[0:D, 0] = ln_scale
    aux[0:D, 1] = ln_shift
    aux[0:D, 2] = mr_bias + EPS
    aux[:, 4] = EPS
    aux[:, 5] = hb

    ln_trivial = bool(np.all(ln_scale == 1.0) and np.all(ln_shift == 0.0))
    mr_trivial = bool(np.all(mr_bias == 0.0))

    nc = build_program(ln_trivial, mr_trivial)

    z0 = np.concatenate([z0r, z0i], axis=1)          # [B, 128]
    in_maps = []
    for c in range(NCORES):
        z0c = np.ascontiguousarray(z0[c * BC:(c + 1) * BC].T)  # [128, BC]
        in_maps.append({
            "z0": z0c, "w2": w2, "cbh": cbh, "adj2": adj2, "cb0": cb0,
            "cb03": cb03, "dect": dect, "aux": aux, "ocst": ocst,
            "idm": idm,
        })

    res = run_bass_kernel_spmd(nc, in_maps, list(range(NCORES)))
    global LAST_EXEC_NS
    LAST_EXEC_NS = res.exec_time_ns
    outs = res.results

    logits = np.concatenate([outs[c]["logits"].T for c in range(NCORES)],
                            axis=0) + dec_b[None, :]
    vq_raw = sum(float(outs[c]["scalars"][0, 0]) for c in range(NCORES))
    pond_raw = sum(float(outs[c]["scalars"][0, 1]) for c in range(NCORES))
    vq_total = np.float32((1.0 + CC) / (B * 2 * D) * vq_raw)
    ponder = np.float32(pond_raw / B + 1.0)   # step 0: still == 1 everywhere
    return logits.astype(f32), ponder, vq_total


if __name__ == "__main__":
    sys.path.insert(0, os.path.dirname(os.path.abspath(__file__)))
    import reference

    inputs = {k: np.asarray(v) for k, v in reference.setup_inputs().items()}
    out = kernel(**inputs)
    print("logits", out[0].shape, out[0].dtype)
    print("ponder", out[1], "vq", out[2])
